# revision 1
# baseline (speedup 1.0000x reference)
"""Data-adaptive weight-ensembling MLP (per-sample expert-merged FFN) on 8 trn2 cores.

Math (per sample b):
  c[b,:,:]  = gate(x)[b].reshape(E, L)          (2-layer relu MLP gate)
  W1[b] = bW1 + sum_e c[b,e,0] tvW1[e];  b1[b] = bb1 + sum_e c[b,e,1] tvb1[e]
  W2[b] = bW2 + sum_e c[b,e,2] tvW2[e];  b2[b] = bb2 + sum_e c[b,e,3] tvb2[e]
  out[b] = relu(x[b] @ W1[b].T + b1[b]) @ W2[b].T + b2[b]

Merged weights are never materialized:
  x[b] @ W1[b].T = x[b] @ bW1.T + sum_e c[b,e,0] (x[b] @ tvW1[e].T)
and the weighted expert sum happens inside PSUM accumulation: for expert e the
matmul stationary operand is X1T[e][d, b] = x[b, d] * c[b, e, 0], so every
task-vector element streams through the PE exactly once.

Sharding (8 cores): DFF=4096 split into 8 slices of 512. Core k computes
layer-1 output columns in its slice (full d-contraction locally -> exact
pre-activation -> local relu), then contracts layer 2 over the same f-slice.
One final AllReduce ([16,1024] = 64KB) sums layer-2 partials. Task-vector
banks are sharded along DFF (64MB/core); gate weights replicated.
"""

import contextlib

import numpy as np

B, D, DFF, E, L = 16, 1024, 4096, 16, 4
NCORES = 8
OSL = DFF // NCORES          # 512: per-core DFF slice
KC1 = D // 128               # 8 k-chunks for the d contraction
KC2 = OSL // 128             # 4 k-chunks for the f contraction

_cache = {}


def _build(reps: int = 1, collective: bool = True, cfg: str = "f32"):
    import concourse.bacc as bacc
    import concourse.bass as bass
    import concourse.tile as tile
    import concourse.mybir as mybir
    from concourse.masks import make_identity

    f32 = mybir.dt.float32
    # wdt: dtype of the big weight streams and their lhsT partners
    # mmcast: AP transform applied to big-matmul operands (f32r bitcast)
    if cfg == "bf16":
        wdt = mybir.dt.bfloat16
        mmcast = lambda ap: ap
    elif cfg == "f32r":
        wdt = f32
        mmcast = lambda ap: ap.bitcast(mybir.dt.float32r)
    else:
        wdt = f32
        mmcast = lambda ap: ap
    Relu = mybir.ActivationFunctionType.Relu
    nc = bacc.Bacc("TRN2", target_bir_lowering=False, debug=False,
                   num_devices=NCORES)

    # ---- I/O (per-core data supplied via in_maps) ----
    xT_h = nc.dram_tensor("xT", [128, KC1, B], wdt, kind="ExternalInput")
    gw1_h = nc.dram_tensor("gw1", [128, KC1, D], wdt, kind="ExternalInput")
    gb1_h = nc.dram_tensor("gb1v", [1, D], f32, kind="ExternalInput")
    gw2_h = nc.dram_tensor("gw2", [128, KC1, E * L], wdt, kind="ExternalInput")
    gb2_h = nc.dram_tensor("gb2v", [1, E * L], f32, kind="ExternalInput")
    tv1_h = nc.dram_tensor("tv1", [E, 128, KC1, OSL], wdt, kind="ExternalInput")
    bw1_h = nc.dram_tensor("bw1", [128, KC1, OSL], wdt, kind="ExternalInput")
    bb1_h = nc.dram_tensor("bb1v", [1, OSL], f32, kind="ExternalInput")
    tvb1_h = nc.dram_tensor("tvb1", [E, OSL], f32, kind="ExternalInput")
    tv2_h = nc.dram_tensor("tv2", [E, 128, KC2, D], wdt, kind="ExternalInput")
    bw2_h = nc.dram_tensor("bw2", [128, KC2, D], wdt, kind="ExternalInput")
    bb2_h = nc.dram_tensor("bb2v", [1, D], f32, kind="ExternalInput")
    tvb2_h = nc.dram_tensor("tvb2", [E, D], f32, kind="ExternalInput")
    out_h = nc.dram_tensor("out", [B, D], f32, kind="ExternalOutput")

    ar_in = nc.dram_tensor("ar_in", [B, D], f32, kind="Internal")
    ar_out = nc.dram_tensor("ar_out", [B, D], f32, kind="Internal",
                            addr_space="Shared")

    with tile.TileContext(nc) as tc, contextlib.ExitStack() as ctx:
        const = ctx.enter_context(tc.tile_pool(name="const", bufs=1))
        small = ctx.enter_context(tc.tile_pool(name="small", bufs=1))
        gwp = ctx.enter_context(tc.tile_pool(name="gwp", bufs=1))
        tvp1 = ctx.enter_context(tc.tile_pool(name="tvp1", bufs=3))
        tvp2 = ctx.enter_context(tc.tile_pool(name="tvp2", bufs=3))
        pacc = ctx.enter_context(tc.tile_pool(name="pacc", bufs=1,
                                              space="PSUM"))
        psml = ctx.enter_context(tc.tile_pool(name="psml", bufs=2,
                                              space="PSUM"))

        # constants (once)
        ones1 = const.tile([1, B], f32)
        nc.vector.memset(ones1[:], 1.0)
        ident16 = const.tile([B, B], f32)
        make_identity(nc, ident16[:])
        ones16_128 = const.tile([B, 128], f32)
        nc.vector.memset(ones16_128[:], 1.0)

        for _rep in range(reps):
            # small inputs
            xT = small.tile([128, KC1, B], wdt, name=f"xT_{_rep}", tag="xT")
            nc.sync.dma_start(out=xT[:], in_=xT_h.ap())
            gb1v = small.tile([1, D], f32, name=f"gb1v_{_rep}", tag="gb1v")
            nc.sync.dma_start(out=gb1v[:], in_=gb1_h.ap())
            gb2v = small.tile([1, E * L], f32, name=f"gb2v_{_rep}", tag="gb2v")
            nc.sync.dma_start(out=gb2v[:], in_=gb2_h.ap())
            bb1v = small.tile([1, OSL], f32, name=f"bb1v_{_rep}", tag="bb1v")
            nc.sync.dma_start(out=bb1v[:], in_=bb1_h.ap())
            tvb1t = small.tile([E, OSL], f32, name=f"tvb1t_{_rep}", tag="tvb1t")
            nc.sync.dma_start(out=tvb1t[:], in_=tvb1_h.ap())
            bb2v = small.tile([1, D], f32, name=f"bb2v_{_rep}", tag="bb2v")
            nc.sync.dma_start(out=bb2v[:], in_=bb2_h.ap())
            tvb2t = small.tile([E, D], f32, name=f"tvb2t_{_rep}", tag="tvb2t")
            nc.sync.dma_start(out=tvb2t[:], in_=tvb2_h.ap())
            gw2t = small.tile([128, KC1, E * L], wdt, name=f"gw2t_{_rep}",
                              tag="gw2t")
            nc.sync.dma_start(out=gw2t[:], in_=gw2_h.ap())
            gw1t = gwp.tile([128, KC1, D], wdt, name=f"gw1t_{_rep}",
                            tag="gw1t")
            nc.sync.dma_start(out=gw1t[:], in_=gw1_h.ap())

            # ---- gate layer 1: g_h = relu(x @ gW1.T + gb1) ----
            g_h = small.tile([B, D], f32, name=f"g_h_{_rep}", tag="g_h")
            for n in range(2):
                gps = pacc.tile([B, 512], f32, tag="gps")
                nc.tensor.matmul(gps[:], ones1[:],
                                 gb1v[:, n * 512:(n + 1) * 512],
                                 start=True, stop=False)
                for kc in range(KC1):
                    nc.tensor.matmul(gps[:], mmcast(xT[:, kc, :]),
                                     mmcast(gw1t[:, kc, n * 512:(n + 1) * 512]),
                                     start=False, stop=(kc == KC1 - 1))
                nc.scalar.activation(g_h[:, n * 512:(n + 1) * 512], gps[:],
                                     Relu)

            # ---- transpose g_h -> ghT [128, (kc, b)] ----
            ghT = small.tile([128, KC1, B], wdt, name=f"ghT_{_rep}", tag="ghT")
            for kc in range(KC1):
                pt = psml.tile([128, B], f32, tag="ps")
                nc.tensor.transpose(pt[:], g_h[:, kc * 128:(kc + 1) * 128],
                                    ident16[:])
                nc.vector.tensor_copy(ghT[:, kc, :], pt[:])

            # ---- gate layer 2: codings; cod[b, e, l] ----
            cps = psml.tile([B, E * L], f32, tag="ps")
            nc.tensor.matmul(cps[:], ones1[:], gb2v[:], start=True, stop=False)
            for kc in range(KC1):
                nc.tensor.matmul(cps[:], mmcast(ghT[:, kc, :]),
                                 mmcast(gw2t[:, kc, :]),
                                 start=False, stop=(kc == KC1 - 1))
            cod = small.tile([B, E, L], f32, name=f"cod_{_rep}", tag="cod")
            nc.vector.tensor_copy(cod[:],
                                  cps[:].rearrange("b (e l) -> b e l", e=E))

            # ---- bias-coefficient matrices cT_l[e, b] = c[b, e, l] ----
            cT = {}
            for l in (1, 3):
                cl = small.tile([B, E], f32, name=f"cl{l}_{_rep}",
                                tag=f"cl{l}")
                nc.vector.tensor_copy(cl[:], cod[:, :, l])
                ptc = psml.tile([B, E], f32, tag="ps")
                nc.tensor.transpose(ptc[:], cl[:], ident16[:])
                cTl = small.tile([E, B], f32, name=f"cT{l}_{_rep}",
                                 tag=f"cT{l}")
                nc.vector.tensor_copy(cTl[:], ptc[:])
                cT[l] = cTl

            # ---- broadcast tiles cbc[l][e][p, b] = c[b, e, l] ----
            cbc = {0: [], 2: []}
            for l in (0, 2):
                for e in range(E):
                    diag = small.tile([B, B], f32, name=f"dg{l}_{e}_{_rep}",
                                      tag="diag")
                    nc.vector.tensor_scalar_mul(diag[:], ident16[:],
                                                cod[:, e, l:l + 1])
                    pb = psml.tile([128, B], f32, tag="ps")
                    nc.tensor.matmul(pb[:], ones16_128[:], diag[:],
                                     start=True, stop=True)
                    bc = small.tile([128, B], wdt, name=f"bc{l}_{e}_{_rep}",
                                    tag=f"bc{l}_{e}")
                    nc.vector.tensor_copy(bc[:], pb[:])
                    cbc[l].append(bc)

            # ---- X1T[e][128, kc, b] = xT * c1[b, e] ----
            x1t = []
            for e in range(E):
                t = small.tile([128, KC1, B], wdt, name=f"x1t{e}_{_rep}",
                               tag=f"x1t{e}")
                nc.vector.tensor_mul(
                    t[:], xT[:],
                    cbc[0][e][:, None, :].broadcast_to([128, KC1, B]))
                x1t.append(t)

            # ---- layer 1: psum1[b, o] = full local pre-activation ----
            psum1 = pacc.tile([B, OSL], f32, tag="psum1")
            nc.tensor.matmul(psum1[:], ones1[:], bb1v[:], start=True,
                             stop=False)
            nc.tensor.matmul(psum1[:], cT[1][:], tvb1t[:], start=False,
                             stop=False)
            for e in range(E + 1):
                tvt = tvp1.tile([128, KC1, OSL], wdt, tag="tvt1")
                nc.sync.dma_start(out=tvt[:],
                                  in_=bw1_h.ap() if e == E else tv1_h.ap()[e])
                lhs = xT if e == E else x1t[e]
                for kc in range(KC1):
                    nc.tensor.matmul(psum1[:], mmcast(lhs[:, kc, :]),
                                     mmcast(tvt[:, kc, :]),
                                     start=False,
                                     stop=(e == E and kc == KC1 - 1))

            h1 = small.tile([B, OSL], f32, name=f"h1_{_rep}", tag="h1")
            nc.scalar.activation(h1[:], psum1[:], Relu)

            # ---- transpose h1 -> h1T [128, (fc, b)] ----
            h1T = small.tile([128, KC2, B], wdt, name=f"h1T_{_rep}", tag="h1T")
            for fc in range(KC2):
                pt2 = psml.tile([128, B], f32, tag="ps")
                nc.tensor.transpose(pt2[:], h1[:, fc * 128:(fc + 1) * 128],
                                    ident16[:])
                nc.vector.tensor_copy(h1T[:, fc, :], pt2[:])

            # ---- X2T[e][128, fc, b] = h1T * c2[b, e] ----
            x2t = []
            for e in range(E):
                t = small.tile([128, KC2, B], wdt, name=f"x2t{e}_{_rep}",
                               tag=f"x2t{e}")
                nc.vector.tensor_mul(
                    t[:], h1T[:],
                    cbc[2][e][:, None, :].broadcast_to([128, KC2, B]))
                x2t.append(t)

            # ---- layer 2: psum2[n][b, j] partial over local f-slice ----
            psum2 = []
            for n in range(2):
                p = pacc.tile([B, 512], f32, tag=f"psum2_{n}")
                nc.tensor.matmul(p[:], ones1[:],
                                 bb2v[:, n * 512:(n + 1) * 512],
                                 start=True, stop=False)
                nc.tensor.matmul(p[:], cT[3][:],
                                 tvb2t[:, n * 512:(n + 1) * 512],
                                 start=False, stop=False)
                psum2.append(p)
            for e in range(E + 1):
                tvt2 = tvp2.tile([128, KC2, D], wdt, tag="tvt2")
                nc.sync.dma_start(out=tvt2[:],
                                  in_=bw2_h.ap() if e == E else tv2_h.ap()[e])
                lhs = h1T if e == E else x2t[e]
                for fc in range(KC2):
                    for n in range(2):
                        nc.tensor.matmul(psum2[n][:], mmcast(lhs[:, fc, :]),
                                         mmcast(tvt2[:, fc, n * 512:(n + 1) * 512]),
                                         start=False,
                                         stop=(e == E and fc == KC2 - 1))

            outp = small.tile([B, D], f32, name=f"outp_{_rep}", tag="outp")
            for n in range(2):
                nc.vector.tensor_copy(outp[:, n * 512:(n + 1) * 512],
                                      psum2[n][:])

            # ---- final AllReduce over all 8 cores ----
            if collective:
                nc.sync.dma_start(out=ar_in.ap(), in_=outp[:])
                nc.gpsimd.collective_compute(
                    "AllReduce", mybir.AluOpType.add,
                    replica_groups=[list(range(NCORES))],
                    ins=[ar_in.ap().opt()],
                    outs=[ar_out.ap().opt()],
                )
                nc.sync.dma_start(out=out_h.ap(), in_=ar_out.ap())
            else:
                nc.sync.dma_start(out=out_h.ap(), in_=outp[:])

    nc.compile()
    return nc


def _prep_inputs(x, gW1, gb1, gW2, gb2, bW1, bb1, bW2, bb2,
                 tvW1, tvb1, tvW2, tvb2, cfg="f32"):
    """Build the 8 per-core in_maps (DMA-friendly layouts)."""
    f = np.float32
    if cfg == "bf16":
        import ml_dtypes
        w = np.dtype(ml_dtypes.bfloat16)
    else:
        w = f
    asf = lambda a: np.ascontiguousarray(a, dtype=f)
    asw = lambda a: np.ascontiguousarray(a.astype(f), dtype=w)

    xT = asw(x.T.reshape(KC1, 128, B).transpose(1, 0, 2))
    gw1 = asw(gW1.T.reshape(KC1, 128, D).transpose(1, 0, 2))
    gw2 = asw(gW2.T.reshape(KC1, 128, E * L).transpose(1, 0, 2))
    gb1v = asf(gb1.reshape(1, D))
    gb2v = asf(gb2.reshape(1, E * L))

    in_maps = []
    for k in range(NCORES):
        o0 = k * OSL
        tv1 = asw(tvW1[:, o0:o0 + OSL, :].transpose(0, 2, 1)
                  .reshape(E, KC1, 128, OSL).transpose(0, 2, 1, 3))
        bw1 = asw(bW1[o0:o0 + OSL, :].T.reshape(KC1, 128, OSL)
                  .transpose(1, 0, 2))
        tv2 = asw(tvW2[:, :, o0:o0 + OSL].transpose(0, 2, 1)
                  .reshape(E, KC2, 128, D).transpose(0, 2, 1, 3))
        bw2 = asw(bW2[:, o0:o0 + OSL].T.reshape(KC2, 128, D)
                  .transpose(1, 0, 2))
        zero = k != 0
        in_maps.append(dict(
            xT=xT, gw1=gw1, gb1v=gb1v, gw2=gw2, gb2v=gb2v,
            tv1=tv1, bw1=bw1,
            bb1v=asf(bb1[o0:o0 + OSL].reshape(1, OSL)),
            tvb1=asf(tvb1[:, o0:o0 + OSL]),
            tv2=tv2, bw2=bw2,
            bb2v=np.zeros((1, D), f) if zero else asf(bb2.reshape(1, D)),
            tvb2=np.zeros((E, D), f) if zero else asf(tvb2),
        ))
    return in_maps


CFG = "bf16"


def kernel(**inputs):
    from concourse.bass_utils import run_bass_kernel_spmd

    key = ("nc", CFG)
    if key not in _cache:
        _cache[key] = _build(cfg=CFG)
    nc = _cache[key]

    in_maps = _prep_inputs(**{k: np.asarray(v) for k, v in inputs.items()},
                           cfg=CFG)
    res = run_bass_kernel_spmd(nc, in_maps, core_ids=list(range(NCORES)))
    return res.results[0]["out"]



# revision 5
# speedup vs baseline: 1.2894x; 1.2894x over previous
"""Data-adaptive weight-ensembling MLP (per-sample expert-merged FFN) on 8 trn2 cores.

Math (per sample b):
  c[b,:,:]  = gate(x)[b].reshape(E, L)          (2-layer relu MLP gate)
  W1[b] = bW1 + sum_e c[b,e,0] tvW1[e];  b1[b] = bb1 + sum_e c[b,e,1] tvb1[e]
  W2[b] = bW2 + sum_e c[b,e,2] tvW2[e];  b2[b] = bb2 + sum_e c[b,e,3] tvb2[e]
  out[b] = relu(x[b] @ W1[b].T + b1[b]) @ W2[b].T + b2[b]

Merged weights are never materialized:
  x[b] @ W1[b].T = x[b] @ bW1.T + sum_e c[b,e,0] (x[b] @ tvW1[e].T)
and the weighted expert sum happens inside PSUM accumulation: for expert e the
matmul stationary operand is X1T[e][d, b] = x[b, d] * c[b, e, 0], so every
task-vector element streams through the PE exactly once.

fp8 cfg: the task-vector banks are stored in e4m3 (pre-scaled by SW), the
stationary x*c stays bf16 (mixed-dtype matmul), and a per-layer residual
stream R = sum_e (tv_e - q(tv_e)) rides along as an 18th matmul with
coefficient mean_e c[b,e] — this cancels the common-mode quantization error.
All layer matmuls (bias/base/experts) accumulate into one PSUM at scale SW;
the final activation rescales by 1/SW.

Sharding (8 cores): DFF=4096 split into 8 slices of 512. Core k computes
layer-1 output columns in its slice (full d-contraction locally -> exact
pre-activation -> local relu), then contracts layer 2 over the same f-slice.
A final ReduceScatter leaves each core with 2 batch rows of the summed
output ([2,1024] = 8KB); the host concatenates the 8 slices. Task-vector
banks are sharded along DFF; gate weights replicated.
"""

import contextlib

import numpy as np

B, D, DFF, E, L = 16, 1024, 4096, 16, 4
NCORES = 8
OSL = DFF // NCORES          # 512: per-core DFF slice
KC1 = D // 128               # 8 k-chunks for the d contraction
KC2 = OSL // 128             # 4 k-chunks for the f contraction
NE = E + 1                   # experts + residual-correction stream
SW1 = 512.0                  # fp8 scale for tv1 (tv std 0.02 -> ~10)
SW2 = 512.0                  # fp8 scale for tv2
RMULT = 8.0                  # extra scale on the R streams

_cache = {}


def _build(reps: int = 1, collective: bool = True, cfg: str = "fp8"):
    import concourse.bacc as bacc
    import concourse.bass as bass
    import concourse.tile as tile
    import concourse.mybir as mybir
    from concourse.masks import make_identity

    f32 = mybir.dt.float32
    bf = mybir.dt.bfloat16
    f8 = mybir.dt.float8e4
    fp8 = cfg == "fp8"
    # wdt: dtype of x-side tiles and (non-fp8) weight streams
    if cfg in ("bf16", "fp8"):
        wdt = bf
        mmcast = lambda ap: ap
    elif cfg == "f32r":
        wdt = f32
        mmcast = lambda ap: ap.bitcast(mybir.dt.float32r)
    else:
        wdt = f32
        mmcast = lambda ap: ap
    tvdt = f8 if fp8 else wdt
    nexp = NE if fp8 else E
    Relu = mybir.ActivationFunctionType.Relu
    Copy = mybir.ActivationFunctionType.Copy
    nc = bacc.Bacc("TRN2", target_bir_lowering=False, debug=False,
                   num_devices=NCORES)

    # ---- I/O (per-core data supplied via in_maps) ----
    xT_h = nc.dram_tensor("xT", [128, KC1, B], wdt, kind="ExternalInput")
    gw1_h = nc.dram_tensor("gw1", [128, KC1, D], wdt, kind="ExternalInput")
    gb1_h = nc.dram_tensor("gb1v", [1, D], f32, kind="ExternalInput")
    gw2_h = nc.dram_tensor("gw2", [128, KC1, E * L], wdt, kind="ExternalInput")
    gb2_h = nc.dram_tensor("gb2v", [1, E * L], f32, kind="ExternalInput")
    tv1_h = nc.dram_tensor("tv1", [nexp, 128, KC1, OSL], tvdt,
                           kind="ExternalInput")
    bw1_h = nc.dram_tensor("bw1", [128, KC1, OSL], wdt, kind="ExternalInput")
    bb1_h = nc.dram_tensor("bb1v", [1, OSL], f32, kind="ExternalInput")
    tvb1_h = nc.dram_tensor("tvb1", [E, OSL], f32, kind="ExternalInput")
    tv2_h = nc.dram_tensor("tv2", [nexp, 128, KC2, D], tvdt,
                           kind="ExternalInput")
    bw2_h = nc.dram_tensor("bw2", [128, KC2, D], wdt, kind="ExternalInput")
    bb2_h = nc.dram_tensor("bb2v", [1, D], f32, kind="ExternalInput")
    tvb2_h = nc.dram_tensor("tvb2", [E, D], f32, kind="ExternalInput")
    if fp8:
        out_h = nc.dram_tensor("out", [B // NCORES, D], f32,
                               kind="ExternalOutput")
    else:
        out_h = nc.dram_tensor("out", [B, D], f32, kind="ExternalOutput")

    ar_in = nc.dram_tensor("ar_in", [B, D], f32, kind="Internal")
    if fp8:
        ar_out = nc.dram_tensor("ar_out", [B // NCORES, D], f32,
                                kind="Internal")
    else:
        ar_out = nc.dram_tensor("ar_out", [B, D], f32, kind="Internal",
                                addr_space="Shared")

    with tile.TileContext(nc) as tc, contextlib.ExitStack() as ctx:
        const = ctx.enter_context(tc.tile_pool(name="const", bufs=1))
        small = ctx.enter_context(tc.tile_pool(name="small", bufs=1))
        gwp = ctx.enter_context(tc.tile_pool(name="gwp", bufs=1))
        basep = ctx.enter_context(tc.tile_pool(name="basep", bufs=1))
        tvp1 = ctx.enter_context(tc.tile_pool(name="tvp1", bufs=4))
        tvp2 = ctx.enter_context(tc.tile_pool(name="tvp2", bufs=4))
        pacc = ctx.enter_context(tc.tile_pool(name="pacc", bufs=1,
                                              space="PSUM"))
        psml = ctx.enter_context(tc.tile_pool(name="psml", bufs=2,
                                              space="PSUM"))

        # constants (once)
        ones1 = const.tile([1, B], f32)
        nc.vector.memset(ones1[:], 1.0)
        ident16 = const.tile([B, B], f32)
        make_identity(nc, ident16[:])
        ones16_128 = const.tile([B, 128], f32)
        nc.vector.memset(ones16_128[:], 1.0)
        if fp8:
            # identity / (E * RMULT): builds the R-stream coefficient
            # (mean_e c[b,e]) / RMULT from a row-sum of codings
            identR = const.tile([B, B], f32)
            make_identity(nc, identR[:])
            nc.vector.tensor_scalar_mul(identR[:], identR[:],
                                        1.0 / (E * RMULT))
            onesE = const.tile([B, E], f32)
            nc.vector.memset(onesE[:], 1.0)

        for _rep in range(reps):
            # small inputs
            xT = small.tile([128, KC1, B], wdt, name=f"xT_{_rep}", tag="xT")
            nc.sync.dma_start(out=xT[:], in_=xT_h.ap())
            gb1v = small.tile([1, D], f32, name=f"gb1v_{_rep}", tag="gb1v")
            nc.sync.dma_start(out=gb1v[:], in_=gb1_h.ap())
            gb2v = small.tile([1, E * L], f32, name=f"gb2v_{_rep}", tag="gb2v")
            nc.sync.dma_start(out=gb2v[:], in_=gb2_h.ap())
            bb1v = small.tile([1, OSL], f32, name=f"bb1v_{_rep}", tag="bb1v")
            nc.sync.dma_start(out=bb1v[:], in_=bb1_h.ap())
            tvb1t = small.tile([E, OSL], f32, name=f"tvb1t_{_rep}", tag="tvb1t")
            nc.sync.dma_start(out=tvb1t[:], in_=tvb1_h.ap())
            bb2v = small.tile([1, D], f32, name=f"bb2v_{_rep}", tag="bb2v")
            nc.sync.dma_start(out=bb2v[:], in_=bb2_h.ap())
            tvb2t = small.tile([E, D], f32, name=f"tvb2t_{_rep}", tag="tvb2t")
            nc.sync.dma_start(out=tvb2t[:], in_=tvb2_h.ap())
            gw2t = small.tile([128, KC1, E * L], wdt, name=f"gw2t_{_rep}",
                              tag="gw2t")
            nc.sync.dma_start(out=gw2t[:], in_=gw2_h.ap())
            gw1t = gwp.tile([128, KC1, D], wdt, name=f"gw1t_{_rep}",
                            tag="gw1t")
            nc.sync.dma_start(out=gw1t[:], in_=gw1_h.ap())
            # base weights prefetch (consumed early in each layer)
            base1 = basep.tile([128, KC1, OSL], wdt, name=f"base1_{_rep}",
                               tag="base1")
            nc.sync.dma_start(out=base1[:], in_=bw1_h.ap())
            base2 = basep.tile([128, KC2, D], wdt, name=f"base2_{_rep}",
                               tag="base2")
            nc.sync.dma_start(out=base2[:], in_=bw2_h.ap())

            # ---- gate layer 1: g_h = relu(x @ gW1.T + gb1) ----
            g_h = small.tile([B, D], f32, name=f"g_h_{_rep}", tag="g_h")
            for n in range(2):
                gps = pacc.tile([B, 512], f32, tag="gps")
                nc.tensor.matmul(gps[:], ones1[:],
                                 gb1v[:, n * 512:(n + 1) * 512],
                                 start=True, stop=False)
                for kc in range(KC1):
                    nc.tensor.matmul(gps[:], mmcast(xT[:, kc, :]),
                                     mmcast(gw1t[:, kc, n * 512:(n + 1) * 512]),
                                     start=False, stop=(kc == KC1 - 1))
                nc.scalar.activation(g_h[:, n * 512:(n + 1) * 512], gps[:],
                                     Relu)

            # ---- transpose g_h -> ghT [128, (kc, b)] ----
            ghT = small.tile([128, KC1, B], wdt, name=f"ghT_{_rep}", tag="ghT")
            for kc in range(KC1):
                pt = psml.tile([128, B], f32, tag="ps")
                nc.tensor.transpose(pt[:], g_h[:, kc * 128:(kc + 1) * 128],
                                    ident16[:])
                nc.vector.tensor_copy(ghT[:, kc, :], pt[:])

            # ---- gate layer 2: codings; cod[b, e, l] ----
            cps = psml.tile([B, E * L], f32, tag="ps")
            nc.tensor.matmul(cps[:], ones1[:], gb2v[:], start=True, stop=False)
            for kc in range(KC1):
                nc.tensor.matmul(cps[:], mmcast(ghT[:, kc, :]),
                                 mmcast(gw2t[:, kc, :]),
                                 start=False, stop=(kc == KC1 - 1))
            cod = small.tile([B, E, L], f32, name=f"cod_{_rep}", tag="cod")
            nc.vector.tensor_copy(cod[:],
                                  cps[:].rearrange("b (e l) -> b e l", e=E))

            # ---- bias-coefficient matrices cT_l[e, b] = c[b, e, l] ----
            cT = {}
            for l in (1, 3):
                cl = small.tile([B, E], f32, name=f"cl{l}_{_rep}",
                                tag=f"cl{l}")
                nc.vector.tensor_copy(cl[:], cod[:, :, l])
                ptc = psml.tile([B, E], f32, tag="ps")
                nc.tensor.transpose(ptc[:], cl[:], ident16[:])
                cTl = small.tile([E, B], f32, name=f"cT{l}_{_rep}",
                                 tag=f"cT{l}")
                nc.vector.tensor_copy(cTl[:], ptc[:])
                cT[l] = cTl

            # ---- per-expert coefficients, incl. R coefficient ----
            # cmean[l][b, 1] = (sum_e c[b,e,l]) / (E*RMULT)
            cmean = {}
            if fp8:
                for l in (0, 2):
                    cm = small.tile([B, 1], f32, name=f"cm{l}_{_rep}",
                                    tag=f"cm{l}")
                    nc.vector.tensor_reduce(cm[:], cod[:, :, l],
                                            axis=mybir.AxisListType.X,
                                            op=mybir.AluOpType.add)
                    cmean[l] = cm

            # ---- broadcast tiles cbc[l][e][p, b] = coeff[b, e] ----
            cbc = {0: [], 2: []}
            for l in (0, 2):
                for e in range(nexp):
                    diag = small.tile([B, B], f32, name=f"dg{l}_{e}_{_rep}",
                                      tag="diag")
                    if fp8 and e == E:
                        nc.vector.tensor_scalar_mul(diag[:], identR[:],
                                                    cmean[l][:, 0:1])
                    else:
                        nc.vector.tensor_scalar_mul(diag[:], ident16[:],
                                                    cod[:, e, l:l + 1])
                    pb = psml.tile([128, B], f32, tag="ps")
                    nc.tensor.matmul(pb[:], ones16_128[:], diag[:],
                                     start=True, stop=True)
                    bc = small.tile([128, B], wdt, name=f"bc{l}_{e}_{_rep}",
                                    tag=f"bc{l}_{e}")
                    nc.vector.tensor_copy(bc[:], pb[:])
                    cbc[l].append(bc)

            # ---- X1T[e][128, kc, b] = xT * c1[b, e] ----
            x1t = []
            for e in range(nexp):
                t = small.tile([128, KC1, B], wdt, name=f"x1t{e}_{_rep}",
                               tag=f"x1t{e}")
                nc.vector.tensor_mul(
                    t[:], xT[:],
                    cbc[0][e][:, None, :].broadcast_to([128, KC1, B]))
                x1t.append(t)

            # ---- layer 1: psum1[b, o] = full local pre-activation (*SW1) ----
            psum1 = pacc.tile([B, OSL], f32, tag="psum1")
            nc.tensor.matmul(psum1[:], ones1[:], bb1v[:], start=True,
                             stop=False)
            nc.tensor.matmul(psum1[:], cT[1][:], tvb1t[:], start=False,
                             stop=False)
            for kc in range(KC1):
                nc.tensor.matmul(psum1[:], mmcast(xT[:, kc, :]),
                                 mmcast(base1[:, kc, :]),
                                 start=False, stop=False)
            for e in range(nexp):
                tvt = tvp1.tile([128, KC1, OSL], tvdt, tag="tvt1")
                nc.sync.dma_start(out=tvt[:], in_=tv1_h.ap()[e])
                for kc in range(KC1):
                    nc.tensor.matmul(psum1[:], mmcast(x1t[e][:, kc, :]),
                                     mmcast(tvt[:, kc, :]),
                                     start=False,
                                     stop=(e == nexp - 1 and kc == KC1 - 1))

            h1 = small.tile([B, OSL], f32, name=f"h1_{_rep}", tag="h1")
            nc.scalar.activation(h1[:], psum1[:], Relu,
                                 scale=(1.0 / SW1) if fp8 else 1.0)

            # ---- transpose h1 -> h1T [128, (fc, b)] ----
            h1T = small.tile([128, KC2, B], wdt, name=f"h1T_{_rep}", tag="h1T")
            for fc in range(KC2):
                pt2 = psml.tile([128, B], f32, tag="ps")
                nc.tensor.transpose(pt2[:], h1[:, fc * 128:(fc + 1) * 128],
                                    ident16[:])
                nc.vector.tensor_copy(h1T[:, fc, :], pt2[:])

            # ---- X2T[e][128, fc, b] = h1T * c2[b, e] ----
            x2t = []
            for e in range(nexp):
                t = small.tile([128, KC2, B], wdt, name=f"x2t{e}_{_rep}",
                               tag=f"x2t{e}")
                nc.vector.tensor_mul(
                    t[:], h1T[:],
                    cbc[2][e][:, None, :].broadcast_to([128, KC2, B]))
                x2t.append(t)

            # ---- layer 2: psum2[n][b, j] partial over local f-slice ----
            psum2 = []
            for n in range(2):
                p = pacc.tile([B, 512], f32, tag=f"psum2_{n}")
                nc.tensor.matmul(p[:], ones1[:],
                                 bb2v[:, n * 512:(n + 1) * 512],
                                 start=True, stop=False)
                nc.tensor.matmul(p[:], cT[3][:],
                                 tvb2t[:, n * 512:(n + 1) * 512],
                                 start=False, stop=False)
                for fc in range(KC2):
                    nc.tensor.matmul(p[:], mmcast(h1T[:, fc, :]),
                                     mmcast(base2[:, fc, n * 512:(n + 1) * 512]),
                                     start=False, stop=False)
                psum2.append(p)
            for e in range(nexp):
                tvt2 = tvp2.tile([128, KC2, D], tvdt, tag="tvt2")
                nc.sync.dma_start(out=tvt2[:], in_=tv2_h.ap()[e])
                for fc in range(KC2):
                    for n in range(2):
                        nc.tensor.matmul(psum2[n][:], mmcast(x2t[e][:, fc, :]),
                                         mmcast(tvt2[:, fc, n * 512:(n + 1) * 512]),
                                         start=False,
                                         stop=(e == nexp - 1 and fc == KC2 - 1))

            outp = small.tile([B, D], f32, name=f"outp_{_rep}", tag="outp")
            for n in range(2):
                nc.scalar.activation(outp[:, n * 512:(n + 1) * 512],
                                     psum2[n][:], Copy,
                                     scale=(1.0 / SW2) if fp8 else 1.0)

            # ---- final collective over all 8 cores ----
            if collective:
                nc.sync.dma_start(out=ar_in.ap(), in_=outp[:])
                if fp8:
                    nc.gpsimd.collective_compute(
                        "ReduceScatter", mybir.AluOpType.add,
                        replica_groups=[list(range(NCORES))],
                        ins=[ar_in.ap().opt()],
                        outs=[ar_out.ap().opt()],
                    )
                else:
                    nc.gpsimd.collective_compute(
                        "AllReduce", mybir.AluOpType.add,
                        replica_groups=[list(range(NCORES))],
                        ins=[ar_in.ap().opt()],
                        outs=[ar_out.ap().opt()],
                    )
                nc.sync.dma_start(out=out_h.ap(), in_=ar_out.ap())
            else:
                nc.sync.dma_start(out=out_h.ap(), in_=outp[:, 0:D] if not fp8
                                  else outp[0:B // NCORES, :])

    nc.compile()
    return nc


def _prep_inputs(x, gW1, gb1, gW2, gb2, bW1, bb1, bW2, bb2,
                 tvW1, tvb1, tvW2, tvb2, cfg="fp8"):
    """Build the 8 per-core in_maps (DMA-friendly layouts)."""
    import ml_dtypes

    f = np.float32
    fp8 = cfg == "fp8"
    if cfg in ("bf16", "fp8"):
        w = np.dtype(ml_dtypes.bfloat16)
    else:
        w = f
    e4 = np.dtype(ml_dtypes.float8_e4m3)
    asf = lambda a: np.ascontiguousarray(a, dtype=f)
    asw = lambda a: np.ascontiguousarray(a.astype(f), dtype=w)
    q8 = lambda a: np.clip(a, -240.0, 240.0).astype(e4)

    xT = asw(x.T.reshape(KC1, 128, B).transpose(1, 0, 2))
    gw1 = asw(gW1.T.reshape(KC1, 128, D).transpose(1, 0, 2))
    gw2 = asw(gW2.T.reshape(KC1, 128, E * L).transpose(1, 0, 2))
    gb1v = asf(gb1.reshape(1, D))
    gb2v = asf(gb2.reshape(1, E * L))

    s1 = SW1 if fp8 else 1.0
    s2 = SW2 if fp8 else 1.0

    in_maps = []
    for k in range(NCORES):
        o0 = k * OSL
        # [E, 512(o), 1024(d)] and [E, 1024(j), 512(f)] slices
        tv1s = tvW1[:, o0:o0 + OSL, :].astype(f) * s1
        tv2s = tvW2[:, :, o0:o0 + OSL].astype(f) * s2
        if fp8:
            tq1 = q8(tv1s)                      # [E, OSL, D]
            r1 = (tv1s.sum(0) - tq1.astype(f).sum(0)) * RMULT
            tv1all = np.concatenate([tq1, q8(r1)[None]], axis=0)
            tq2 = q8(tv2s)
            r2 = (tv2s.sum(0) - tq2.astype(f).sum(0)) * RMULT
            tv2all = np.concatenate([tq2, q8(r2)[None]], axis=0)
            cvt = lambda a: np.ascontiguousarray(a)
            nexp = NE
        else:
            tv1all, tv2all = tv1s, tv2s
            cvt = lambda a: np.ascontiguousarray(a.astype(f), dtype=w)
            nexp = E
        tv1 = cvt(tv1all.transpose(0, 2, 1)
                  .reshape(nexp, KC1, 128, OSL).transpose(0, 2, 1, 3))
        tv2 = cvt(tv2all.transpose(0, 2, 1)
                  .reshape(nexp, KC2, 128, D).transpose(0, 2, 1, 3))
        bw1 = asw((bW1[o0:o0 + OSL, :].astype(f) * s1).T
                  .reshape(KC1, 128, OSL).transpose(1, 0, 2))
        bw2 = asw((bW2[:, o0:o0 + OSL].astype(f) * s2).T
                  .reshape(KC2, 128, D).transpose(1, 0, 2))
        zero = k != 0
        in_maps.append(dict(
            xT=xT, gw1=gw1, gb1v=gb1v, gw2=gw2, gb2v=gb2v,
            tv1=tv1, bw1=bw1,
            bb1v=asf(bb1[o0:o0 + OSL].reshape(1, OSL) * s1),
            tvb1=asf(tvb1[:, o0:o0 + OSL] * s1),
            tv2=tv2, bw2=bw2,
            bb2v=np.zeros((1, D), f) if zero else asf(bb2.reshape(1, D) * s2),
            tvb2=np.zeros((E, D), f) if zero else asf(tvb2 * s2),
        ))
    return in_maps


CFG = "fp8"


def kernel(**inputs):
    from concourse.bass_utils import run_bass_kernel_spmd

    key = ("nc", CFG)
    if key not in _cache:
        _cache[key] = _build(cfg=CFG)
    nc = _cache[key]

    in_maps = _prep_inputs(**{k: np.asarray(v) for k, v in inputs.items()},
                           cfg=CFG)
    res = run_bass_kernel_spmd(nc, in_maps, core_ids=list(range(NCORES)))
    if CFG == "fp8":
        return np.concatenate([res.results[k]["out"] for k in range(NCORES)],
                              axis=0)
    return res.results[0]["out"]
